# revision 4
# baseline (speedup 1.0000x reference)
"""Min-max normalization kernel for Trainium2 (Bass/Tile), SPMD over 8 cores.

Problem: x of shape (16, 12, 32, 128, 128) f32. For each (i, j, k) slice of
shape (128, 128): out = (x - min) / (max - min + 1e-8), min/max over the slice.

Strategy: flatten to (6144, 16384) — 6144 independent slices of 16384 elements.
Pure data-parallel over 8 cores: 768 slices per core, viewed as 6 groups of
128 slices. Each slice lives on one SBUF partition, so min/max is a free-dim
reduce on the Vector engine, and the normalize is one fused
(x - min) * inv tensor_scalar per chunk. Memory-bound: ~100 MB HBM traffic
per core (50 MB read + 50 MB write).
"""

import numpy as np

N_CORES = 8
P = 128              # partitions = slices per group
FREE = 16384         # 128*128 elements per slice
GROUPS = 6           # groups per core: 768 slices / 128
CHUNK = 4096         # free-dim chunk per DMA/compute op
NCHUNK = FREE // CHUNK
EPS = 1e-8
FULL_SHAPE = (16, 12, 32, 128, 128)

_nc_cache = {}


def _build_nc():
    import concourse.bacc as bacc
    import concourse.tile as tile
    from concourse import mybir

    f32 = mybir.dt.float32
    nc = bacc.Bacc(None, target_bir_lowering=False)
    x = nc.dram_tensor("x", [GROUPS, P, FREE], f32, kind="ExternalInput")
    y = nc.dram_tensor("y", [GROUPS, P, FREE], f32, kind="ExternalOutput")

    with tile.TileContext(nc) as tc:
        with tc.tile_pool(name="data", bufs=10) as data, \
             tc.tile_pool(name="stats", bufs=3) as stats, \
             tc.tile_pool(name="scal", bufs=3) as scal:
            for g in range(GROUPS):
                pmax = stats.tile([P, NCHUNK], f32, tag="pmax")
                pmin = stats.tile([P, NCHUNK], f32, tag="pmin")
                chunks = []
                for c in range(NCHUNK):
                    t = data.tile([P, CHUNK], f32, tag="data")
                    nc.gpsimd.dma_start(
                        out=t[:, :], in_=x[g, :, c * CHUNK:(c + 1) * CHUNK]
                    )
                    nc.vector.tensor_reduce(
                        out=pmax[:, c:c + 1], in_=t[:, :],
                        axis=mybir.AxisListType.X, op=mybir.AluOpType.max,
                    )
                    nc.vector.tensor_reduce(
                        out=pmin[:, c:c + 1], in_=t[:, :],
                        axis=mybir.AxisListType.X, op=mybir.AluOpType.min,
                    )
                    chunks.append(t)

                rmax = scal.tile([P, 1], f32, tag="rmax")
                rmin = scal.tile([P, 1], f32, tag="rmin")
                inv = scal.tile([P, 1], f32, tag="inv")
                nc.vector.tensor_reduce(
                    out=rmax[:, :], in_=pmax[:, :],
                    axis=mybir.AxisListType.X, op=mybir.AluOpType.max,
                )
                nc.vector.tensor_reduce(
                    out=rmin[:, :], in_=pmin[:, :],
                    axis=mybir.AxisListType.X, op=mybir.AluOpType.min,
                )
                # inv = 1 / (rmax - rmin + EPS)
                nc.vector.tensor_scalar(
                    out=inv[:, :], in0=rmax[:, :],
                    scalar1=rmin[:, 0:1], scalar2=EPS,
                    op0=mybir.AluOpType.subtract, op1=mybir.AluOpType.add,
                )
                nc.vector.reciprocal(out=inv[:, :], in_=inv[:, :])

                for c, t in enumerate(chunks):
                    # out = (x - rmin) * inv, in place
                    nc.vector.tensor_scalar(
                        out=t[:, :], in0=t[:, :],
                        scalar1=rmin[:, 0:1], scalar2=inv[:, 0:1],
                        op0=mybir.AluOpType.subtract, op1=mybir.AluOpType.mult,
                    )
                    nc.gpsimd.dma_start(
                        out=y[g, :, c * CHUNK:(c + 1) * CHUNK], in_=t[:, :]
                    )
    nc.compile()
    return nc


def _get_nc():
    if "nc" not in _nc_cache:
        _nc_cache["nc"] = _build_nc()
    return _nc_cache["nc"]


def run(x: np.ndarray, trace: bool = False):
    """Shard, run on 8 cores, gather. Returns (out, BassKernelResults)."""
    from concourse.bass_utils import run_bass_kernel_spmd

    x = np.asarray(x, dtype=np.float32)
    assert x.shape == FULL_SHAPE, x.shape
    xs = x.reshape(N_CORES, GROUPS, P, FREE)
    in_maps = [{"x": np.ascontiguousarray(xs[c])} for c in range(N_CORES)]
    nc = _get_nc()
    res = run_bass_kernel_spmd(nc, in_maps, core_ids=list(range(N_CORES)),
                               trace=trace)
    out = np.stack([res.results[c]["y"] for c in range(N_CORES)])
    return out.reshape(FULL_SHAPE), res


def kernel(**inputs) -> np.ndarray:
    out, _ = run(inputs["x"], trace=False)
    return out


# revision 10
# speedup vs baseline: 2.2630x; 2.2630x over previous
"""Min-max normalization kernel for Trainium2 (Bass/Tile), SPMD over 8 cores.

Problem: x of shape (16, 12, 32, 128, 128) f32. For each (i, j, k) slice of
shape (128, 128): out = (x - min) / (max - min + 1e-8), min/max over the slice.

Strategy: flatten to (6144, 16384) — 6144 independent slices of 16384 elements.
Pure data-parallel over 8 cores: 768 slices per core, viewed as 6 groups of
128 slices. Each slice lives on one SBUF partition, so min/max is a free-dim
reduce on the Vector engine, and the normalize is one fused
(x - min) * inv tensor_scalar per chunk. Memory-bound: ~100 MB HBM traffic
per core (50 MB read + 50 MB write).
"""

import numpy as np

N_CORES = 8
P = 128              # partitions = slices per group
FREE = 16384         # 128*128 elements per slice
GROUPS = 6           # groups per core: 768 slices / 128
CHUNK = 4096         # free-dim chunk per DMA/compute op
NCHUNK = FREE // CHUNK
EPS = 1e-8
FULL_SHAPE = (16, 12, 32, 128, 128)

_nc_cache = {}


def _build_nc(chunk=CHUNK, bufs=11, load_eng="gpsimd", store_eng="sync",
              repeat=1):
    import concourse.bacc as bacc
    import concourse.tile as tile
    from concourse import mybir

    nchunk = FREE // chunk
    f32 = mybir.dt.float32
    nc = bacc.Bacc(None, target_bir_lowering=False)
    x = nc.dram_tensor("x", [GROUPS, P, FREE], f32, kind="ExternalInput")
    y = nc.dram_tensor("y", [GROUPS, P, FREE], f32, kind="ExternalOutput")
    load = getattr(nc, load_eng)
    store = getattr(nc, store_eng)

    with tile.TileContext(nc) as tc:
        with tc.tile_pool(name="data", bufs=bufs) as data, \
             tc.tile_pool(name="stats", bufs=3) as stats, \
             tc.tile_pool(name="scal", bufs=3) as scal:
            for g in [g for _ in range(repeat) for g in range(GROUPS)]:
                pmax = stats.tile([P, nchunk], f32, tag="pmax")
                pmin = stats.tile([P, nchunk], f32, tag="pmin")
                chunks = []
                for c in range(nchunk):
                    t = data.tile([P, chunk], f32, tag="data")
                    load.dma_start(
                        out=t[:, :], in_=x[g, :, c * chunk:(c + 1) * chunk]
                    )
                    nc.vector.tensor_reduce(
                        out=pmax[:, c:c + 1], in_=t[:, :],
                        axis=mybir.AxisListType.X, op=mybir.AluOpType.max,
                    )
                    nc.vector.tensor_reduce(
                        out=pmin[:, c:c + 1], in_=t[:, :],
                        axis=mybir.AxisListType.X, op=mybir.AluOpType.min,
                    )
                    chunks.append(t)

                rmax = scal.tile([P, 1], f32, tag="rmax")
                rmin = scal.tile([P, 1], f32, tag="rmin")
                inv = scal.tile([P, 1], f32, tag="inv")
                nbias = scal.tile([P, 1], f32, tag="nbias")
                nc.vector.tensor_reduce(
                    out=rmax[:, :], in_=pmax[:, :],
                    axis=mybir.AxisListType.X, op=mybir.AluOpType.max,
                )
                nc.vector.tensor_reduce(
                    out=rmin[:, :], in_=pmin[:, :],
                    axis=mybir.AxisListType.X, op=mybir.AluOpType.min,
                )
                # inv = 1 / (rmax - rmin + EPS)
                nc.vector.tensor_scalar(
                    out=inv[:, :], in0=rmax[:, :],
                    scalar1=rmin[:, 0:1], scalar2=EPS,
                    op0=mybir.AluOpType.subtract, op1=mybir.AluOpType.add,
                )
                nc.vector.reciprocal(out=inv[:, :], in_=inv[:, :])
                # nbias = -rmin * inv
                nc.vector.tensor_scalar(
                    out=nbias[:, :], in0=rmin[:, :],
                    scalar1=inv[:, 0:1], scalar2=-1.0,
                    op0=mybir.AluOpType.mult, op1=mybir.AluOpType.mult,
                )

                for c, t in enumerate(chunks):
                    # out = x * inv + (-rmin * inv), in place, on ACT (keeps
                    # DVE free for the reduces; DMA stays the bottleneck)
                    nc.scalar.activation(
                        out=t[:, :], in_=t[:, :],
                        func=mybir.ActivationFunctionType.Identity,
                        bias=nbias[:, 0:1], scale=inv[:, 0:1],
                    )
                    store.dma_start(
                        out=y[g, :, c * chunk:(c + 1) * chunk], in_=t[:, :]
                    )
    nc.compile()
    return nc


def _get_nc():
    if "nc" not in _nc_cache:
        _nc_cache["nc"] = _build_nc()
    return _nc_cache["nc"]


def run(x: np.ndarray, trace: bool = False):
    """Shard, run on 8 cores, gather. Returns (out, BassKernelResults)."""
    from concourse.bass_utils import run_bass_kernel_spmd

    x = np.asarray(x, dtype=np.float32)
    assert x.shape == FULL_SHAPE, x.shape
    xs = x.reshape(N_CORES, GROUPS, P, FREE)
    in_maps = [{"x": np.ascontiguousarray(xs[c])} for c in range(N_CORES)]
    nc = _get_nc()
    res = run_bass_kernel_spmd(nc, in_maps, core_ids=list(range(N_CORES)),
                               trace=trace)
    out = np.stack([res.results[c]["y"] for c in range(N_CORES)])
    return out.reshape(FULL_SHAPE), res


def kernel(**inputs) -> np.ndarray:
    out, _ = run(inputs["x"], trace=False)
    return out


# revision 12
# speedup vs baseline: 2.3160x; 1.0234x over previous
"""Min-max normalization kernel for Trainium2 (Bass/Tile), SPMD over 8 cores.

Problem: x of shape (16, 12, 32, 128, 128) f32. For each (i, j, k) slice of
shape (128, 128): out = (x - min) / (max - min + 1e-8), min/max over the slice.

Strategy: flatten to (6144, 16384) — 6144 independent slices of 16384 elements.
Pure data-parallel over 8 cores: 768 slices per core, viewed as 6 groups of
128 slices. Each slice lives on one SBUF partition, so min/max is a free-dim
reduce on the Vector engine, and the normalize is one fused
(x - min) * inv tensor_scalar per chunk. Memory-bound: ~100 MB HBM traffic
per core (50 MB read + 50 MB write).
"""

import numpy as np

N_CORES = 8
P = 128              # partitions = slices per group
FREE = 16384         # 128*128 elements per slice
GROUPS = 6           # groups per core: 768 slices / 128
CHUNK = 4096         # free-dim chunk per DMA/compute op
NCHUNK = FREE // CHUNK
EPS = 1e-8
FULL_SHAPE = (16, 12, 32, 128, 128)

_nc_cache = {}


def _build_nc(chunk=CHUNK, bufs=11, load_eng="gpsimd", store_eng="sync",
              repeat=1):
    import concourse.bacc as bacc
    import concourse.tile as tile
    from concourse import mybir

    nchunk = FREE // chunk
    f32 = mybir.dt.float32
    nc = bacc.Bacc(None, target_bir_lowering=False)
    x = nc.dram_tensor("x", [GROUPS, P, FREE], f32, kind="ExternalInput")
    y = nc.dram_tensor("y", [GROUPS, P, FREE], f32, kind="ExternalOutput")
    load = getattr(nc, load_eng)
    store = getattr(nc, store_eng)

    with tile.TileContext(nc) as tc:
        with tc.tile_pool(name="data", bufs=bufs) as data, \
             tc.tile_pool(name="stats", bufs=3) as stats, \
             tc.tile_pool(name="scal", bufs=3) as scal:
            for gi, g in enumerate(
                    [g for _ in range(repeat) for g in range(GROUPS)]):
                pmax = stats.tile([P, nchunk], f32, tag="pmax")
                pmin = stats.tile([P, nchunk], f32, tag="pmin")
                chunks = []
                for c in range(nchunk):
                    t = data.tile([P, chunk], f32, tag="data")
                    # The very first load goes out on HWDGE (~0.6 us first
                    # byte vs ~2.4 us SWDGE descgen) to cut the lead-in.
                    ld = store if gi == 0 and c == 0 else load
                    ld.dma_start(
                        out=t[:, :], in_=x[g, :, c * chunk:(c + 1) * chunk]
                    )
                    nc.vector.tensor_reduce(
                        out=pmax[:, c:c + 1], in_=t[:, :],
                        axis=mybir.AxisListType.X, op=mybir.AluOpType.max,
                    )
                    nc.vector.tensor_reduce(
                        out=pmin[:, c:c + 1], in_=t[:, :],
                        axis=mybir.AxisListType.X, op=mybir.AluOpType.min,
                    )
                    chunks.append(t)

                rmax = scal.tile([P, 1], f32, tag="rmax")
                rmin = scal.tile([P, 1], f32, tag="rmin")
                inv = scal.tile([P, 1], f32, tag="inv")
                nbias = scal.tile([P, 1], f32, tag="nbias")
                nc.vector.tensor_reduce(
                    out=rmax[:, :], in_=pmax[:, :],
                    axis=mybir.AxisListType.X, op=mybir.AluOpType.max,
                )
                nc.vector.tensor_reduce(
                    out=rmin[:, :], in_=pmin[:, :],
                    axis=mybir.AxisListType.X, op=mybir.AluOpType.min,
                )
                # inv = 1 / (rmax - rmin + EPS)
                nc.vector.tensor_scalar(
                    out=inv[:, :], in0=rmax[:, :],
                    scalar1=rmin[:, 0:1], scalar2=EPS,
                    op0=mybir.AluOpType.subtract, op1=mybir.AluOpType.add,
                )
                nc.vector.reciprocal(out=inv[:, :], in_=inv[:, :])
                # nbias = -rmin * inv
                nc.vector.tensor_scalar(
                    out=nbias[:, :], in0=rmin[:, :],
                    scalar1=inv[:, 0:1], scalar2=-1.0,
                    op0=mybir.AluOpType.mult, op1=mybir.AluOpType.mult,
                )

                for c, t in enumerate(chunks):
                    # out = x * inv + (-rmin * inv), in place, on ACT (keeps
                    # DVE free for the reduces; DMA stays the bottleneck)
                    nc.scalar.activation(
                        out=t[:, :], in_=t[:, :],
                        func=mybir.ActivationFunctionType.Identity,
                        bias=nbias[:, 0:1], scale=inv[:, 0:1],
                    )
                    store.dma_start(
                        out=y[g, :, c * chunk:(c + 1) * chunk], in_=t[:, :]
                    )
    nc.compile()
    return nc


def _get_nc():
    if "nc" not in _nc_cache:
        _nc_cache["nc"] = _build_nc()
    return _nc_cache["nc"]


def run(x: np.ndarray, trace: bool = False):
    """Shard, run on 8 cores, gather. Returns (out, BassKernelResults)."""
    from concourse.bass_utils import run_bass_kernel_spmd

    x = np.asarray(x, dtype=np.float32)
    assert x.shape == FULL_SHAPE, x.shape
    xs = x.reshape(N_CORES, GROUPS, P, FREE)
    in_maps = [{"x": np.ascontiguousarray(xs[c])} for c in range(N_CORES)]
    nc = _get_nc()
    res = run_bass_kernel_spmd(nc, in_maps, core_ids=list(range(N_CORES)),
                               trace=trace)
    out = np.stack([res.results[c]["y"] for c in range(N_CORES)])
    return out.reshape(FULL_SHAPE), res


def kernel(**inputs) -> np.ndarray:
    out, _ = run(inputs["x"], trace=False)
    return out
